# revision 59
# baseline (speedup 1.0000x reference)
"""nn_LSTETransformer kernel for 8 trn2 NeuronCores.

The axon tunnel (~45-55 MB/s each way, half-duplex) dominates the end-to-end
time, so the design minimizes and overlaps wire traffic:

- Transformer body on-device, tensor-parallel per the hint: 2 heads per core
  (Wq/Wk/Wv row-shards), Wo column-shard + f16 AllReduce per block; MLP wg/wu
  row-shards of 512, wd column-shard + f16 AllReduce; embedding build is
  token-sharded (256 tokens/core) and AllGathered on device.
- Hybrid LM head: the device computes vocab rows [0, V_DEV) as token-major
  int8 with a per-row scale, emitted in NW wave outputs; the host computes
  rows [V_DEV, V) with a BLAS sgemm from the downloaded final-norm
  activations (f16, one replicated shard), overlapped with the int8 wave
  downloads, which are widened between GEMM blocks as they land.
- Uploads are staged: blob A (emb/attention/norms) ships while the host still
  packs blob B (MLP + LM codes); host-side LM dequant runs during device
  exec. Ternary codes ship packed 4-per-byte (u8 modular-arithmetic pack) and
  are unpacked + group-dequantized (f16 scales) on device; weight/probability
  transposes use the DMA xbar transpose (out[p,g,q] = in[q,g*128+p]).
- Everything (Bass build, XLA/NEFF compile, device load) is warmed at import
  with a zero-input run; big host buffers are preallocated and pre-touched.

Self-contained: only imports numpy/jax/concourse (on sys.path here).
"""

import numpy as np

try:  # persistent XLA compilation cache saves a few seconds per fresh process
    import jax as _jax
    _jax.config.update("jax_compilation_cache_dir", "/root/.jax_comp_cache")
    _jax.config.update("jax_persistent_cache_min_entry_size_bytes", -1)
    _jax.config.update("jax_persistent_cache_min_compile_time_secs", 0)
except Exception:
    pass

import concourse.bass as bass
import concourse.mybir as mybir
import concourse.tile as tile
from concourse.bass_utils import run_bass_kernel_spmd

N_CORES = 8
B, S, D, H, DFF, V, L = 2, 1024, 1024, 16, 4096, 32000, 4
GS = 128
DH = D // H            # 64
TOK = B * S            # 2048
VSH = V // N_CORES     # 4000
KT = D // 128          # 8 feature tiles
FSH = DFF // N_CORES   # 512 ff rows per core
HL = H // N_CORES      # 2 heads per core

V_DEV = 20480          # vocab rows computed on device (host does the rest)
VSH_D = V_DEV // N_CORES  # per core, VSH_D/128 chunks
V_HOST = V - V_DEV     # 13568

f32 = mybir.dt.float32
f16 = mybir.dt.float16
i8 = mybir.dt.int8
u8 = mybir.dt.uint8
AF = mybir.ActivationFunctionType
ALU = mybir.AluOpType
AX = mybir.AxisListType

LAST_EXEC_NS = None

# ---------------------------------------------------------------- device part


def _split_excess_waits(nc, max_waits=1):
    """walrus here rejects >1 sem-wait per instruction; hoist extras onto NOPs."""
    for fn in nc.m.functions:
        for blk in fn.blocks:
            new_insts, dirty = [], False
            for inst in blk.instructions:
                si = inst.sync_info
                if si is not None and si.on_wait and len(si.on_wait) > max_waits:
                    waits = list(si.on_wait)
                    excess, keep = waits[:-max_waits], waits[-max_waits:]
                    for i in range(0, len(excess), max_waits):
                        new_insts.append(
                            mybir.InstNoOp(
                                name=f"{inst.name}-waitsplit-{i}",
                                engine=inst.engine,
                                sync_info=mybir.SyncInfo(
                                    on_wait=excess[i : i + max_waits], on_update=[]
                                ),
                                text_hint="waitsplit",
                                bass_nofuse=True,
                            )
                        )
                    inst.sync_info = mybir.SyncInfo(
                        on_wait=keep, on_update=list(si.on_update)
                    )
                    dirty = True
                new_insts.append(inst)
            if dirty:
                blk.instructions = new_insts


def _rmsnorm(nc, tc, epsb, xT, hT, ncol, ones_col, ones_row):
    """hT[:,k,t] = xT[:,k,t] * rsqrt(mean_D(x^2)+eps) * ncol[:,k] (f16 out)."""
    with (
        tc.tile_pool(name="nrm_sb", bufs=2) as nsb,
        tc.tile_pool(name="nrm_ps", bufs=1, space="PSUM") as nps,
    ):
        ssq = nps.tile([1, 4, 512], f32)
        for k in range(KT):
            sq = nsb.tile([128, 2048], f16, tag="sq")
            nc.vector.tensor_mul(out=sq[:], in0=xT[:, k, :], in1=xT[:, k, :])
            for ch in range(4):
                nc.tensor.matmul(
                    out=ssq[:, ch, :],
                    lhsT=ones_col[:],
                    rhs=sq[:, ch * 512 : (ch + 1) * 512],
                    start=(k == 0),
                    stop=(k == KT - 1),
                )
        rms = nsb.tile([1, 2048], f32, tag="rms")
        nc.scalar.activation(
            out=rms[:], in_=ssq[:].rearrange("p a b -> p (a b)"),
            func=AF.Sqrt, bias=epsb[:1, :], scale=1.0 / D,
        )
        rrow = nsb.tile([1, 2048], f32, tag="rrow")
        nc.vector.reciprocal(out=rrow[:], in_=rms[:])
        bc = nps.tile([128, 4, 512], f32)
        for ch in range(4):
            nc.tensor.matmul(
                out=bc[:, ch, :],
                lhsT=ones_row[:],
                rhs=rrow[:, ch * 512 : (ch + 1) * 512],
                start=True, stop=True,
            )
        rmsb = nsb.tile([128, 2048], f16, tag="rmsb")
        nc.scalar.copy(out=rmsb[:], in_=bc[:].rearrange("p a b -> p (a b)"))
        nc.vector.tensor_tensor(
            out=hT[:], in0=xT[:],
            in1=rmsb[:].unsqueeze(1).broadcast_to([128, KT, 2048]),
            op=ALU.mult,
        )
        nc.vector.tensor_tensor(
            out=hT[:], in0=hT[:],
            in1=ncol[:].unsqueeze(2).broadcast_to([128, KT, 2048]),
            op=ALU.mult,
        )


def _allreduce_add(nc, tc, dram, name, psum_feed, xT):
    """psum_feed(ot) -> psum [128,2048] f32; AllReduce over cores; xT += result.
    The collective runs in f16 (halves the reduced bytes; partials are O(1))."""
    arin = dram.tile([KT, 128, 2048], f16, tag=f"{name}_in")
    arout = dram.tile([KT, 128, 2048], f16, tag=f"{name}_out", addr_space="Shared")
    with tc.tile_pool(name=f"{name}_st", bufs=2) as stp:
        for ot in range(KT):
            ps = psum_feed(ot)
            st = stp.tile([128, 2048], f16, tag="st")
            nc.scalar.copy(out=st[:], in_=ps)
            nc.sync.dma_start(out=arin[ot, :, :], in_=st[:])
        nc.gpsimd.collective_compute(
            "AllReduce", ALU.add,
            replica_groups=[list(range(N_CORES))],
            ins=[arin[:]], outs=[arout[:]],
        )
        for ot in range(KT):
            rd = stp.tile([128, 2048], f16, tag="st")
            nc.sync.dma_start(out=rd[:], in_=arout[ot, :, :])
            nc.vector.tensor_add(out=xT[:, ot, :], in0=xT[:, ot, :], in1=rd[:])


def _build_nc():
    nc = bass.Bass(num_devices=N_CORES)
    TSH = TOK // N_CORES  # 256 tokens per core for the embedding build
    emb_c = nc.declare_dram_parameter("emb_c", [TSH, D // 4], u8, isOutput=False)
    emb_sc = nc.declare_dram_parameter("emb_sc", [TSH, KT], f16, isOutput=False)
    wqkv = nc.declare_dram_parameter("wqkv", [L, 3, 128, D // 4], u8, isOutput=False)
    wqkv_s = nc.declare_dram_parameter("wqkv_s", [L, 3, 128, KT], f16, isOutput=False)
    wo = nc.declare_dram_parameter("wo", [L, D, 32], u8, isOutput=False)
    wo_s = nc.declare_dram_parameter("wo_s", [L, D], f16, isOutput=False)
    wgu = nc.declare_dram_parameter("wgu", [L, 2, FSH, D // 4], u8, isOutput=False)
    wgu_s = nc.declare_dram_parameter("wgu_s", [L, 2, FSH, KT], f16, isOutput=False)
    wd = nc.declare_dram_parameter("wd", [L, D, FSH // 4], u8, isOutput=False)
    wd_s = nc.declare_dram_parameter("wd_s", [L, D, FSH // GS], f16, isOutput=False)
    na = nc.declare_dram_parameter("na", [L, D], f32, isOutput=False)
    nm = nc.declare_dram_parameter("nm", [L, D], f32, isOutput=False)
    fn = nc.declare_dram_parameter("fn", [D], f32, isOutput=False)
    alpha_p = nc.declare_dram_parameter("alpha_p", [L, 128], f32, isOutput=False)
    sel_p = nc.declare_dram_parameter("sel_p", [128, KT], f32, isOutput=False)
    mask_p = nc.declare_dram_parameter("mask_p", [128, 128], f32, isOutput=False)
    lm_t = nc.declare_dram_parameter("lm_t", [VSH_D, D // 4], u8, isOutput=False)
    lm_s = nc.declare_dram_parameter("lm_s", [VSH_D, KT], f16, isOutput=False)
    NW = 4                 # logits download waves
    WCH = VSH_D // NW // 128  # chunks per wave
    WCOLS = WCH * 128      # 768 cols per wave
    outqs = [
        nc.declare_dram_parameter(f"logitsq{w}", [TOK, WCOLS], i8, isOutput=True)
        for w in range(NW)
    ]
    outs = nc.declare_dram_parameter("logscl", [VSH_D, 1], f32, isOutput=True)
    outx = nc.declare_dram_parameter("xnorm", [TOK, D], f16, isOutput=True)

    with tile.TileContext(nc) as tc:
        with (
            tc.tile_pool(name="persist", bufs=1) as pp,
            tc.tile_pool(name="dram", bufs=2, space="DRAM") as dram,
        ):
            xT = pp.tile([128, KT, TOK], f32)
            hT = pp.tile([128, KT, TOK], f16)
            mask = pp.tile([128, 128], f32)
            nc.sync.dma_start(out=mask[:], in_=mask_p[:, :])
            sel = pp.tile([128, KT], f32)
            nc.sync.dma_start(out=sel[:], in_=sel_p[:, :])
            ones_col = pp.tile([128, 1], f16)
            nc.vector.memset(ones_col[:], 1.0)
            ones_row = pp.tile([1, 128], f32)
            nc.vector.memset(ones_row[:], 1.0)
            epsb = pp.tile([128, 1], f32)
            nc.vector.memset(epsb[:], 1e-6)
            shamt = pp.tile([128, 4], u8)
            for _pos in range(4):
                nc.vector.memset(shamt[:, _pos : _pos + 1], 2 * _pos)
            three = pp.tile([128, 1], u8)
            nc.vector.memset(three[:], 3)

            # embedding: each core dequants its 256 tokens, AllGather the rest
            agin = dram.tile([128, KT, TSH], f16, tag="emb_agin")
            agout = dram.tile(
                [N_CORES, 128, KT, TSH], f16, tag="emb_agout", addr_space="Shared"
            )
            with tc.tile_pool(name="emb", bufs=2) as ebp:
                eT = ebp.tile([128, KT, TSH], f16, tag="eT")
                for tt in range(TSH // 128):
                    ec = ebp.tile([128, D // 4], u8, tag="ec")
                    nc.sync.dma_start(
                        out=ec[:], in_=emb_c[tt * 128 : (tt + 1) * 128, :]
                    )
                    ecu = ebp.tile([128, D], u8, tag="ecu")
                    for pos in range(4):
                        nc.vector.tensor_scalar(
                            out=ecu[:, pos * 256 : (pos + 1) * 256], in0=ec[:],
                            scalar1=shamt[:, pos : pos + 1], scalar2=three[:],
                            op0=ALU.logical_shift_right, op1=ALU.bitwise_and,
                        )
                    es = ebp.tile([128, KT], f16, tag="es")
                    nc.sync.dma_start(
                        out=es[:], in_=emb_sc[tt * 128 : (tt + 1) * 128, :]
                    )
                    edq = ebp.tile([128, D], f16, tag="edq")
                    esb = es[:].unsqueeze(2).broadcast_to([128, KT, 128])
                    nc.vector.tensor_tensor(
                        out=edq[:].rearrange("p (g k) -> p g k", g=KT),
                        in0=ecu[:].rearrange("p (g k) -> p g k", g=KT),
                        in1=esb, op=ALU.mult,
                    )
                    nc.vector.tensor_tensor(
                        out=edq[:].rearrange("p (g k) -> p g k", g=KT),
                        in0=edq[:].rearrange("p (g k) -> p g k", g=KT),
                        in1=esb, op=ALU.subtract,
                    )
                    nc.sync.dma_start_transpose(
                        out=eT[:, :, tt * 128 : (tt + 1) * 128], in_=edq[:]
                    )
                nc.sync.dma_start(out=agin[:], in_=eT[:])
                nc.gpsimd.collective_compute(
                    "AllGather", ALU.bypass,
                    replica_groups=[list(range(N_CORES))],
                    ins=[agin[:]], outs=[agout[:]],
                )
                for c in range(N_CORES):
                    nc.sync.dma_start(
                        out=hT[:, :, c * TSH : (c + 1) * TSH], in_=agout[c]
                    )
            nc.vector.tensor_copy(out=xT[:], in_=hT[:])


            for li in range(L):
                # ---- attention block ----
                ncol = pp.tile([128, KT], f32, tag="ncol", bufs=2)
                nc.sync.dma_start(
                    out=ncol[:], in_=na[li, :].rearrange("(k p) -> p k", p=128)
                )
                _rmsnorm(nc, tc, epsb, xT, hT, ncol, ones_col, ones_row)

                with (
                    tc.tile_pool(name="att_sb", bufs=1) as asb,
                    tc.tile_pool(name="att_w", bufs=1) as awp,
                ):
                    # qkv weights -> WjT [128, KT, 128] per j
                    codes = awp.tile([128, 3, D // 4], u8, tag="c")
                    nc.sync.dma_start(
                        out=codes[:], in_=wqkv[li].rearrange("j p f -> p j f")
                    )
                    ucod = awp.tile([128, 3, D], u8, tag="uc")
                    for pos in range(4):
                        nc.vector.tensor_scalar(
                            out=ucod[:, :, pos * 256 : (pos + 1) * 256],
                            in0=codes[:],
                            scalar1=shamt[:, pos : pos + 1], scalar2=three[:],
                            op0=ALU.logical_shift_right, op1=ALU.bitwise_and,
                        )
                    scl = awp.tile([128, 3, KT], f16, tag="s")
                    nc.sync.dma_start(
                        out=scl[:], in_=wqkv_s[li].rearrange("j p f -> p j f")
                    )
                    wdq = awp.tile([128, 3, D], f16, tag="dq")
                    sclb = scl[:].unsqueeze(3).broadcast_to([128, 3, KT, 128])
                    nc.vector.tensor_tensor(
                        out=wdq[:].rearrange("p j (g k) -> p j g k", g=KT),
                        in0=ucod[:].rearrange("p j (g k) -> p j g k", g=KT),
                        in1=sclb, op=ALU.mult,
                    )
                    nc.vector.tensor_tensor(
                        out=wdq[:].rearrange("p j (g k) -> p j g k", g=KT),
                        in0=wdq[:].rearrange("p j (g k) -> p j g k", g=KT),
                        in1=sclb, op=ALU.subtract,
                    )
                    wT = asb.tile([128, 3, KT, 128], f16)
                    for j in range(3):
                        nc.sync.dma_start_transpose(
                            out=wT[:, j, :, :], in_=wdq[:, j, :]
                        )

                    qT = asb.tile([128, TOK], f16)
                    kTt = asb.tile([128, TOK], f16)
                    vT = asb.tile([128, TOK], f16)
                    with tc.tile_pool(name="qkv_ps", bufs=2, space="PSUM") as qps:
                        for j, dst in enumerate((qT, kTt, vT)):
                            ps = qps.tile([128, 4, 512], f32, tag="ps")
                            for ch in range(4):
                                for k in range(KT):
                                    nc.tensor.matmul(
                                        out=ps[:, ch, :],
                                        lhsT=wT[:, j, k, :],
                                        rhs=hT[:, k, ch * 512 : (ch + 1) * 512],
                                        start=(k == 0), stop=(k == KT - 1),
                                    )
                            if j == 0:
                                nc.scalar.mul(
                                    out=dst[:].rearrange("p (a b) -> p a b", a=4),
                                    in_=ps[:], mul=DH ** -0.5,
                                )
                            else:
                                nc.scalar.copy(
                                    out=dst[:].rearrange("p (a b) -> p a b", a=4),
                                    in_=ps[:],
                                )

                    # v natural layout per (b, h): v_nat[pair][k_loc, kt, dh]
                    v_nat = asb.tile([128, 2 * HL, KT, DH], f16)
                    for b in range(B):
                        for h in range(HL):
                            nc.sync.dma_start_transpose(
                                out=v_nat[:, b * HL + h, :, :],
                                in_=vT[h * DH : (h + 1) * DH,
                                       b * S : (b + 1) * S],
                            )

                    oT = asb.tile([128, TOK], f16)
                    with (
                        tc.tile_pool(name="sc_ps", bufs=1, space="PSUM") as scps,
                        tc.tile_pool(name="pv_ps", bufs=2, space="PSUM") as pvps,
                        tc.tile_pool(name="p_sb", bufs=2) as psb,
                    ):
                        for qi in range(8):
                            kext = (qi + 1) * 128
                            for b in range(B):
                                psc = scps.tile([128, 2, 1024], f32, tag="psc")
                                q0 = b * S + qi * 128
                                for h in range(HL):
                                    for c0 in range(0, kext, 512):
                                        cw = min(512, kext - c0)
                                        nc.tensor.matmul(
                                            out=psc[:, h, c0 : c0 + cw],
                                            lhsT=qT[h * DH : (h + 1) * DH,
                                                    q0 : q0 + 128],
                                            rhs=kTt[h * DH : (h + 1) * DH,
                                                    b * S + c0 : b * S + c0 + cw],
                                            start=True, stop=True,
                                        )
                                nc.vector.tensor_tensor(
                                    out=psc[:, :, qi * 128 : kext],
                                    in0=psc[:, :, qi * 128 : kext],
                                    in1=mask[:].unsqueeze(1).broadcast_to(
                                        [128, 2, 128]),
                                    op=ALU.add,
                                )
                                mx = psb.tile([128, 2], f32, tag="mx")
                                nc.vector.tensor_reduce(
                                    out=mx[:], in_=psc[:, :, :kext],
                                    axis=AX.X, op=ALU.max,
                                )
                                nc.vector.tensor_tensor(
                                    out=psc[:, :, :kext], in0=psc[:, :, :kext],
                                    in1=mx[:].unsqueeze(2).broadcast_to(
                                        [128, 2, kext]),
                                    op=ALU.subtract,
                                )
                                pex = psb.tile([128, 2, 1024], f16, tag="pex")
                                nc.scalar.activation(
                                    out=pex[:, :, :kext], in_=psc[:, :, :kext],
                                    func=AF.Exp,
                                )
                                sme = psb.tile([128, 2], f32, tag="sme")
                                nc.vector.tensor_reduce(
                                    out=sme[:], in_=pex[:, :, :kext],
                                    axis=AX.X, op=ALU.add,
                                )
                                rec = psb.tile([128, 2], f32, tag="rec")
                                nc.vector.reciprocal(out=rec[:], in_=sme[:])
                                nc.vector.tensor_tensor(
                                    out=pex[:, :, :kext], in0=pex[:, :, :kext],
                                    in1=rec[:].unsqueeze(2).broadcast_to(
                                        [128, 2, kext]),
                                    op=ALU.mult,
                                )
                                for h in range(HL):
                                    pT = psb.tile([128, 8, 128], f16, tag="pT")
                                    nc.sync.dma_start_transpose(
                                        out=pT[:, : qi + 1, :],
                                        in_=pex[:, h, :kext],
                                    )
                                    pv = pvps.tile([DH, 128], f32, tag="pv")
                                    for kk in range(qi + 1):
                                        nc.tensor.matmul(
                                            out=pv[:],
                                            lhsT=v_nat[:, b * HL + h, kk, :],
                                            rhs=pT[:, kk, :],
                                            start=(kk == 0), stop=(kk == qi),
                                        )
                                    nc.scalar.copy(
                                        out=oT[h * DH : (h + 1) * DH,
                                               q0 : q0 + 128],
                                        in_=pv[:],
                                    )

                    # alpha residual: oT += alpha_col * h_block(core)
                    acol = asb.tile([128, 1], f32)
                    nc.sync.dma_start(out=acol[:], in_=alpha_p[li, :].unsqueeze(1))
                    halp = asb.tile([128, TOK], f16)
                    nc.vector.tensor_scalar_mul(halp[:], hT[:, 0, :], sel[:, 0:1])
                    for k in range(1, KT):
                        nc.vector.scalar_tensor_tensor(
                            out=halp[:], in0=hT[:, k, :], scalar=sel[:, k : k + 1],
                            in1=halp[:], op0=ALU.mult, op1=ALU.add,
                        )
                    nc.vector.scalar_tensor_tensor(
                        out=oT[:], in0=halp[:], scalar=acol[:], in1=oT[:],
                        op0=ALU.mult, op1=ALU.add,
                    )

                    # O-projection partials -> AllReduce -> x update
                    ocodes = awp.tile([128, KT, 32], u8, tag="c")
                    nc.sync.dma_start(
                        out=ocodes[:],
                        in_=wo[li].rearrange("(ot p) i -> p ot i", p=128),
                    )
                    oucod = awp.tile([128, KT, 128], u8, tag="uc")
                    for pos in range(4):
                        nc.vector.tensor_scalar(
                            out=oucod[:, :, pos * 32 : (pos + 1) * 32],
                            in0=ocodes[:],
                            scalar1=shamt[:, pos : pos + 1], scalar2=three[:],
                            op0=ALU.logical_shift_right, op1=ALU.bitwise_and,
                        )
                    oscl = awp.tile([128, KT], f16, tag="s")
                    nc.sync.dma_start(
                        out=oscl[:], in_=wo_s[li].rearrange("(ot p) -> p ot", p=128)
                    )
                    odq = awp.tile([128, KT, 128], f16, tag="dq")
                    osclb = oscl[:].unsqueeze(2).broadcast_to([128, KT, 128])
                    nc.vector.tensor_tensor(
                        out=odq[:], in0=oucod[:], in1=osclb, op=ALU.mult,
                    )
                    nc.vector.tensor_tensor(
                        out=odq[:], in0=odq[:], in1=osclb, op=ALU.subtract,
                    )
                    woT = asb.tile([128, KT, 128], f16)
                    for ot in range(KT):
                        nc.sync.dma_start_transpose(
                            out=woT[:, ot, :], in_=odq[:, ot, :]
                        )
                    with tc.tile_pool(name="op_ps", bufs=2, space="PSUM") as ops:
                        def feed_o(ot):
                            ps = ops.tile([128, 4, 512], f32, tag="ps")
                            for ch in range(4):
                                nc.tensor.matmul(
                                    out=ps[:, ch, :],
                                    lhsT=woT[:, ot, :],
                                    rhs=oT[:, ch * 512 : (ch + 1) * 512],
                                    start=True, stop=True,
                                )
                            return ps[:].rearrange("p a b -> p (a b)")
                        _allreduce_add(nc, tc, dram, f"ar_o{li}", feed_o, xT)

                # ---- MLP block ----
                ncol2 = pp.tile([128, KT], f32, tag="ncol", bufs=2)
                nc.sync.dma_start(
                    out=ncol2[:], in_=nm[li, :].rearrange("(k p) -> p k", p=128)
                )
                _rmsnorm(nc, tc, epsb, xT, hT, ncol2, ones_col, ones_row)

                with (
                    tc.tile_pool(name="mlp_sb", bufs=1) as msb,
                    tc.tile_pool(name="mlp_w", bufs=1) as mwp,
                ):
                    guT = msb.tile([128, 2, KT, FSH], f16)
                    for j in range(2):
                        gcodes = mwp.tile([128, 4, D // 4], u8, tag="c")
                        nc.sync.dma_start(
                            out=gcodes[:],
                            in_=wgu[li, j].rearrange("(ot p) f -> p ot f", p=128),
                        )
                        gucod = mwp.tile([128, 4, D], u8, tag="uc")
                        for pos in range(4):
                            nc.vector.tensor_scalar(
                                out=gucod[:, :, pos * 256 : (pos + 1) * 256],
                                in0=gcodes[:],
                                scalar1=shamt[:, pos : pos + 1], scalar2=three[:],
                                op0=ALU.logical_shift_right, op1=ALU.bitwise_and,
                            )
                        gscl = mwp.tile([128, 4, KT], f16, tag="s")
                        nc.sync.dma_start(
                            out=gscl[:],
                            in_=wgu_s[li, j].rearrange("(ot p) f -> p ot f", p=128),
                        )
                        gdq = mwp.tile([128, 4, D], f16, tag="dq")
                        gsclb = gscl[:].unsqueeze(3).broadcast_to([128, 4, KT, 128])
                        nc.vector.tensor_tensor(
                            out=gdq[:].rearrange("p o (g k) -> p o g k", g=KT),
                            in0=gucod[:].rearrange("p o (g k) -> p o g k", g=KT),
                            in1=gsclb, op=ALU.mult,
                        )
                        nc.vector.tensor_tensor(
                            out=gdq[:].rearrange("p o (g k) -> p o g k", g=KT),
                            in0=gdq[:].rearrange("p o (g k) -> p o g k", g=KT),
                            in1=gsclb, op=ALU.subtract,
                        )
                        for ot in range(4):
                            nc.sync.dma_start_transpose(
                                out=guT[:, j, :, ot * 128 : (ot + 1) * 128],
                                in_=gdq[:, ot, :],
                            )
                    gT = msb.tile([128, 4, TOK], f16)
                    uT = msb.tile([128, 4, TOK], f16)
                    with tc.tile_pool(name="gu_ps", bufs=2, space="PSUM") as gps:
                        for j, dst in enumerate((gT, uT)):
                            for mb in range(4):
                                ps = gps.tile([128, 4, 512], f32, tag="ps")
                                for ch in range(4):
                                    for k in range(KT):
                                        nc.tensor.matmul(
                                            out=ps[:, ch, :],
                                            lhsT=guT[:, j, k,
                                                     mb * 128 : (mb + 1) * 128],
                                            rhs=hT[:, k, ch * 512 : (ch + 1) * 512],
                                            start=(k == 0), stop=(k == KT - 1),
                                        )
                                nc.scalar.copy(
                                    out=dst[:, mb, :].rearrange(
                                        "p (a b) -> p a b", a=4),
                                    in_=ps[:],
                                )
                    nc.scalar.activation(out=gT[:], in_=gT[:], func=AF.Silu)
                    nc.vector.tensor_mul(out=gT[:], in0=gT[:], in1=uT[:])

                    dcodes = mwp.tile([128, KT, FSH // 4], u8, tag="c")
                    nc.sync.dma_start(
                        out=dcodes[:],
                        in_=wd[li].rearrange("(ot p) m -> p ot m", p=128),
                    )
                    ducod = mwp.tile([128, KT, FSH], u8, tag="uc")
                    for pos in range(4):
                        nc.vector.tensor_scalar(
                            out=ducod[:, :, pos * 128 : (pos + 1) * 128],
                            in0=dcodes[:],
                            scalar1=shamt[:, pos : pos + 1], scalar2=three[:],
                            op0=ALU.logical_shift_right, op1=ALU.bitwise_and,
                        )
                    dscl = mwp.tile([128, KT, 4], f16, tag="s")
                    nc.sync.dma_start(
                        out=dscl[:],
                        in_=wd_s[li].rearrange("(ot p) g -> p ot g", p=128),
                    )
                    ddq = mwp.tile([128, KT, FSH], f16, tag="dq")
                    dsclb = dscl[:].unsqueeze(3).broadcast_to([128, KT, 4, 128])
                    nc.vector.tensor_tensor(
                        out=ddq[:].rearrange("p o (g k) -> p o g k", g=4),
                        in0=ducod[:].rearrange("p o (g k) -> p o g k", g=4),
                        in1=dsclb, op=ALU.mult,
                    )
                    nc.vector.tensor_tensor(
                        out=ddq[:].rearrange("p o (g k) -> p o g k", g=4),
                        in0=ddq[:].rearrange("p o (g k) -> p o g k", g=4),
                        in1=dsclb, op=ALU.subtract,
                    )
                    wdT = msb.tile([128, 4, KT, 128], f16)
                    for ot in range(KT):
                        nc.sync.dma_start_transpose(
                            out=wdT[:, :, ot, :], in_=ddq[:, ot, :]
                        )
                    with tc.tile_pool(name="dn_ps", bufs=2, space="PSUM") as dps:
                        def feed_d(ot):
                            ps = dps.tile([128, 4, 512], f32, tag="ps")
                            for ch in range(4):
                                for k in range(4):
                                    nc.tensor.matmul(
                                        out=ps[:, ch, :],
                                        lhsT=wdT[:, k, ot, :],
                                        rhs=gT[:, k, ch * 512 : (ch + 1) * 512],
                                        start=(k == 0), stop=(k == 3),
                                    )
                            return ps[:].rearrange("p a b -> p (a b)")
                        _allreduce_add(nc, tc, dram, f"ar_d{li}", feed_d, xT)

            # ---- final norm + LM head (device vocab part) ----
            fcol = pp.tile([128, KT], f32, tag="ncol", bufs=2)
            nc.sync.dma_start(out=fcol[:], in_=fn[:].rearrange("(k p) -> p k", p=128))
            _rmsnorm(nc, tc, epsb, xT, hT, fcol, ones_col, ones_row)

            # ship token-major xnorm (f16) for the host's vocab part
            with tc.tile_pool(name="xn_sb", bufs=2) as xnp:
                for k in range(KT):
                    tx = xnp.tile([128, 16, 128], f16, tag="tx")
                    nc.sync.dma_start_transpose(out=tx[:], in_=hT[:, k, :])
                    nc.sync.dma_start(
                        out=outx.rearrange("(tg p) d -> p tg d", p=128)[
                            :, :, k * 128 : (k + 1) * 128
                        ],
                        in_=tx[:],
                    )

            with (
                tc.tile_pool(name="lm_w", bufs=3) as lwp,
                tc.tile_pool(name="lm_o", bufs=3) as lop,
                tc.tile_pool(name="lm_ps", bufs=2, space="PSUM") as lps,
            ):
                nvc = VSH_D // 128
                for vc in range(nvc):
                    r0 = vc * 128
                    lcodes = lwp.tile([128, D // 4], u8, tag="lm_c")
                    nc.sync.dma_start(out=lcodes[:], in_=lm_t[r0 : r0 + 128, :])
                    lucod = lwp.tile([128, D], u8, tag="lm_uc")
                    for pos in range(4):
                        nc.vector.tensor_scalar(
                            out=lucod[:, pos * 256 : (pos + 1) * 256],
                            in0=lcodes[:],
                            scalar1=shamt[:, pos : pos + 1], scalar2=three[:],
                            op0=ALU.logical_shift_right, op1=ALU.bitwise_and,
                        )
                    lscl = lwp.tile([128, KT], f16, tag="lm_s")
                    nc.sync.dma_start(out=lscl[:], in_=lm_s[r0 : r0 + 128, :])
                    ldq = lwp.tile([128, D], f16, tag="lm_dq")
                    lsclb = lscl[:].unsqueeze(2).broadcast_to([128, KT, 128])
                    nc.vector.tensor_tensor(
                        out=ldq[:].rearrange("p (g k) -> p g k", g=KT),
                        in0=lucod[:].rearrange("p (g k) -> p g k", g=KT),
                        in1=lsclb, op=ALU.mult,
                    )
                    nc.vector.tensor_tensor(
                        out=ldq[:].rearrange("p (g k) -> p g k", g=KT),
                        in0=ldq[:].rearrange("p (g k) -> p g k", g=KT),
                        in1=lsclb, op=ALU.subtract,
                    )
                    lmT = lwp.tile([128, KT, 128], f16, tag="lm_T")
                    nc.sync.dma_start_transpose(out=lmT[:], in_=ldq[:])
                    ps = lps.tile([128, 4, 512], f32, tag="ps")
                    for ch in range(4):
                        for k in range(KT):
                            nc.tensor.matmul(
                                out=ps[:, ch, :],
                                lhsT=lmT[:, k, :],
                                rhs=hT[:, k, ch * 512 : (ch + 1) * 512],
                                start=(k == 0), stop=(k == KT - 1),
                            )
                    absm = lop.tile([128, 1], f32, tag="absm")
                    lmn = lop.tile([128, 1], f32, tag="lmn")
                    nc.vector.tensor_reduce(
                        out=absm[:], in_=ps[:], axis=AX.XY, op=ALU.max,
                    )
                    nc.vector.tensor_reduce(
                        out=lmn[:], in_=ps[:], axis=AX.XY, op=ALU.min,
                    )
                    nc.scalar.mul(out=lmn[:], in_=lmn[:], mul=-1.0)
                    nc.vector.tensor_max(out=absm[:], in0=absm[:], in1=lmn[:])
                    nc.vector.tensor_scalar_max(absm[:], absm[:], 1e-20)
                    rsc = lop.tile([128, 1], f32, tag="rsc")
                    nc.vector.reciprocal(out=rsc[:], in_=absm[:])
                    nc.vector.tensor_scalar_mul(rsc[:], rsc[:], 127.0)
                    qf = lop.tile([128, TOK], f16, tag="qf")
                    nc.vector.tensor_scalar_mul(
                        qf[:].rearrange("p (a b) -> p a b", a=4),
                        ps[:], rsc[:],
                    )
                    # token-major int8 so the host dequant is contiguous
                    tq = lop.tile([128, 16, 128], f16, tag="tq")
                    nc.sync.dma_start_transpose(out=tq[:], in_=qf[:])
                    ti = lop.tile([128, 16, 128], i8, tag="ti")
                    nc.vector.tensor_copy(out=ti[:], in_=tq[:])
                    rl = (vc % WCH) * 128
                    nc.sync.dma_start(
                        out=outqs[vc // WCH].rearrange("(tg p) v -> p tg v", p=128)[
                            :, :, rl : rl + 128
                        ],
                        in_=ti[:],
                    )
                    osc = lop.tile([128, 1], f32, tag="osc")
                    nc.scalar.mul(out=osc[:], in_=absm[:], mul=1.0 / 127.0)
                    nc.sync.dma_start(out=outs[r0 : r0 + 128, :], in_=osc[:])

    _split_excess_waits(nc)
    return nc


_NC_CACHE = None


def _get_nc():
    global _NC_CACHE
    if _NC_CACHE is None:
        _NC_CACHE = _build_nc()
    return _NC_CACHE


# ----------------------------------------------------------------- host part


def _pack2(c):
    """Pack ternary codes 4-per-byte along the last axis, quarter-interleaved:
    byte j holds codes at last-axis positions {j, q+j, 2q+j, 3q+j} (q = N/4),
    so the device unpack (shift 2*pos) writes contiguous quarters.
    Uses in-place u8 modular arithmetic: (((c3<<2)+c2)<<2+c1)<<2+c0+85."""
    c = np.asarray(c)
    if c.dtype != np.int8:
        c = c.astype(np.int8)
    v = c.view(np.uint8)
    q = v.shape[-1] // 4
    r = np.empty(v.shape[:-1] + (q,), np.uint8)
    np.left_shift(v[..., 3 * q :], 2, out=r)
    r += v[..., 2 * q : 3 * q]
    np.left_shift(r, 2, out=r)
    r += v[..., q : 2 * q]
    np.left_shift(r, 2, out=r)
    r += v[..., :q]
    r += 85
    return r


GROUP_B = ("wgu", "wgu_s", "wd", "wd_s", "lm_t", "lm_s")


def _prep_a(inputs):
    """Group-A per-core maps (emb, qkv, wo, norms): fast, uploaded first."""
    gi = lambda k: np.asarray(inputs[k])
    ids = gi("input_ids").reshape(-1)                        # [2048]
    emb_t = gi("emb_t")
    emb_s = gi("emb_s").astype(np.float32, copy=False).reshape(V, KT)
    emb_c = _pack2(emb_t[ids])                               # [2048, 256] u8
    emb_sc = np.ascontiguousarray(emb_s[ids]).astype(np.float16)

    wo_t = gi("wo_t")
    wq_s = gi("wq_s").astype(np.float32, copy=False).reshape(L, D, KT)
    wk_s = gi("wk_s").astype(np.float32, copy=False).reshape(L, D, KT)
    wv_s = gi("wv_s").astype(np.float32, copy=False).reshape(L, D, KT)
    wo_s = gi("wo_s").astype(np.float32, copy=False).reshape(L, D, KT)
    na_w = gi("na_w").astype(np.float32, copy=False)
    nm_w = gi("nm_w").astype(np.float32, copy=False)
    fn_w = gi("fn_w").astype(np.float32, copy=False)
    alpha = gi("alpha").astype(np.float32, copy=False)       # [L, H]

    mask = np.where(
        np.arange(128)[None, :] <= np.arange(128)[:, None], 0.0, -1e30
    ).astype(np.float32)

    wq_p, wk_p, wv_p = _pack2(gi("wq_t")), _pack2(gi("wk_t")), _pack2(gi("wv_t"))

    maps = []
    for c in range(N_CORES):
        r1 = slice(c * 128, (c + 1) * 128)
        sel = np.zeros((128, KT), np.float32)
        sel[:, c] = 1.0
        acol = alpha[:, HL * c : HL * (c + 1)].repeat(DH, axis=1)  # [L, 128]
        rt = slice(c * (TOK // N_CORES), (c + 1) * (TOK // N_CORES))
        maps.append({
            "emb_c": emb_c[rt], "emb_sc": emb_sc[rt],
            "wqkv": np.stack([wq_p[:, r1], wk_p[:, r1], wv_p[:, r1]], axis=1),
            "wqkv_s": np.stack(
                [wq_s[:, r1], wk_s[:, r1], wv_s[:, r1]], axis=1
            ).astype(np.float16),
            "wo": _pack2(wo_t[:, :, r1]),
            "wo_s": np.ascontiguousarray(wo_s[:, :, c]).astype(np.float16),
            "na": na_w, "nm": nm_w, "fn": fn_w,
            "alpha_p": np.ascontiguousarray(acol),
            "sel_p": sel, "mask_p": mask,
        })
    return maps


def _prep_b(inputs):
    """Group-B per-core maps (MLP + LM): packed while group A uploads."""
    gi = lambda k: np.asarray(inputs[k])
    wd_t = gi("wd_t")
    wg_s = gi("wg_s").astype(np.float32, copy=False).reshape(L, DFF, KT)
    wu_s = gi("wu_s").astype(np.float32, copy=False).reshape(L, DFF, KT)
    wd_s = gi("wd_s").astype(np.float32, copy=False).reshape(L, D, DFF // GS)
    lm_raw = gi("lm_t")                                      # int8 [V, D]
    lm_s = gi("lm_s").astype(np.float32, copy=False).reshape(V, KT)
    _LM_HOST["t"] = lm_raw[V_DEV:]
    _LM_HOST["s"] = lm_s[V_DEV:]

    wg_p, wu_p = _pack2(gi("wg_t")), _pack2(gi("wu_t"))
    lm_p = _pack2(lm_raw[:V_DEV])

    maps = []
    for c in range(N_CORES):
        rf = slice(c * FSH, (c + 1) * FSH)
        rv = slice(c * VSH_D, (c + 1) * VSH_D)
        maps.append({
            "wgu": np.stack([wg_p[:, rf], wu_p[:, rf]], axis=1),
            "wgu_s": np.stack([wg_s[:, rf], wu_s[:, rf]], axis=1).astype(np.float16),
            "wd": _pack2(wd_t[:, :, rf]),
            "wd_s": np.ascontiguousarray(
                wd_s[:, :, 4 * c : 4 * (c + 1)]
            ).astype(np.float16),
            "lm_t": lm_p[rv], "lm_s": lm_s[rv].astype(np.float16),
        })
    return maps


# ----------------------------------------------------------------- runner


_SPMD_CACHE = None


def _get_spmd(nc, n_cores):
    """Build (once) the jitted sharded callable + metadata for nc."""
    global _SPMD_CACHE
    if _SPMD_CACHE is not None:
        return _SPMD_CACHE
    import jax
    import jax.numpy as jnp
    from jax.sharding import Mesh, NamedSharding, PartitionSpec
    from jax.experimental.shard_map import shard_map
    from concourse import bass2jax

    bass2jax.install_neuronx_cc_hook()
    assert nc.dbg_addr is None or not nc.dbg_callbacks
    partition_name = nc.partition_id_tensor.name if nc.partition_id_tensor else None
    in_specs_tbl, out_names, out_avals = [], [], []
    for alloc in nc.m.functions[0].allocations:
        if not isinstance(alloc, mybir.MemoryLocationSet):
            continue
        name = alloc.memorylocations[0].name
        if alloc.kind == "ExternalInput":
            if name != partition_name:
                in_specs_tbl.append(
                    (name, tuple(alloc.tensor_shape), mybir.dt.np(alloc.dtype))
                )
        elif alloc.kind == "ExternalOutput":
            out_names.append(name)
            out_avals.append(
                jax.core.ShapedArray(
                    tuple(alloc.tensor_shape), mybir.dt.np(alloc.dtype)
                )
            )
    in_names = [t[0] for t in in_specs_tbl]
    n_outs = len(out_avals)
    bind_names = list(in_names) + list(out_names)
    if partition_name is not None:
        bind_names.append(partition_name)

    # two upload groups: A uploads while B is still being packed on the host
    specs_a = [t for t in in_specs_tbl if t[0] not in GROUP_B]
    specs_b = [t for t in in_specs_tbl if t[0] in GROUP_B]

    def _make_split(specs):
        def _split(blob):
            # carve the per-core input params out of the fused u8 blob
            from jax import lax

            arrs = []
            off = 0
            for _, shape, dt in specs:
                isz = np.dtype(dt).itemsize
                nb = int(np.prod(shape)) * isz
                seg = blob[off : off + nb]
                if dt == np.uint8:
                    arr = seg.reshape(shape)
                elif isz == 1:
                    arr = lax.bitcast_convert_type(seg, dt).reshape(shape)
                else:
                    arr = lax.bitcast_convert_type(
                        seg.reshape(-1, isz), dt
                    ).reshape(shape)
                arrs.append(arr)
                off += nb
            return tuple(arrs)

        return _split

    def _body(*args):
        operands = list(args)
        if partition_name is not None:
            operands.append(bass2jax.partition_id_tensor())
        outs = bass2jax._bass_exec_p.bind(
            *operands,
            out_avals=tuple(out_avals),
            in_names=tuple(bind_names),
            out_names=tuple(out_names),
            lowering_input_output_aliases=(),
            sim_require_finite=True,
            sim_require_nnan=True,
            nc=nc,
        )
        return tuple(outs)

    devices = jax.devices()[:n_cores]
    mesh = Mesh(np.asarray(devices), ("core",))
    spec = PartitionSpec("core")
    n_params = len(in_names)
    donate = tuple(range(n_params, n_params + n_outs))
    sharded = jax.jit(
        shard_map(
            _body, mesh=mesh, in_specs=(spec,) * (n_params + n_outs),
            out_specs=(spec,) * n_outs, check_rep=False,
        ),
        donate_argnums=donate, keep_unused=True,
    )
    sh = NamedSharding(mesh, spec)
    zero_fn = jax.jit(
        lambda: tuple(
            jnp.zeros((n_cores * a.shape[0], *a.shape[1:]), a.dtype)
            for a in out_avals
        ),
        out_shardings=(sh,) * n_outs,
    )

    split_a = jax.jit(
        shard_map(
            _make_split(specs_a), mesh=mesh, in_specs=(spec,),
            out_specs=(spec,) * len(specs_a), check_rep=False,
        )
    )
    split_b = jax.jit(
        shard_map(
            _make_split(specs_b), mesh=mesh, in_specs=(spec,),
            out_specs=(spec,) * len(specs_b), check_rep=False,
        )
    )
    _SPMD_CACHE = (
        sharded, zero_fn, (split_a, split_b),
        ([t[0] for t in specs_a], [t[0] for t in specs_b]),
        in_names, out_names, sh,
    )
    return _SPMD_CACHE


def _blob(maps, names):
    return np.concatenate(
        [
            np.ascontiguousarray(np.asarray(m[name])).view(np.uint8).reshape(-1)
            for m in maps
            for name in names
        ]
    )


def _run_spmd_staged(nc, maps_a, prep_b_fn):
    """Stage uploads: blob A ships while the host still packs group B (the
    axon tunnel is the bottleneck, so host pack time hides under the wire).
    Zero output buffers are created on-device; everything is async — blocking
    happens when outputs are fetched."""
    import jax
    import time as _time

    _t0 = _time.perf_counter()
    sharded, zero_fn, (split_a, split_b), (names_a, names_b), in_names, \
        out_names, sh = _get_spmd(nc, N_CORES)
    dblob_a = jax.device_put(_blob(maps_a, names_a), sh)  # first wire bytes ASAP
    din_a = split_a(dblob_a)
    dzeros = zero_fn()  # async; zeros materialize during the upload
    TIMINGS["stage_a"] = _time.perf_counter() - _t0
    _t0 = _time.perf_counter()
    maps_b = prep_b_fn()
    TIMINGS["prep_b"] = _time.perf_counter() - _t0
    _t0 = _time.perf_counter()
    dblob_b = jax.device_put(_blob(maps_b, names_b), sh)
    din_b = split_b(dblob_b)
    by_name = dict(zip(names_a, din_a))
    by_name.update(zip(names_b, din_b))
    out_arrs = sharded(*(by_name[n] for n in in_names), *dzeros)
    TIMINGS["stage_b"] = _time.perf_counter() - _t0
    import os as _os

    if _os.environ.get("KBENCH"):
        _DIN.clear()
        _DIN.extend([by_name, in_names])
    return {name: out_arrs[i] for i, name in enumerate(out_names)}


# ----------------------------------------------------------------- entry


TIMINGS = {}
_DIN = []
_LM_HOST = {}

try:  # warm BLAS thread/buffer setup so the first in-kernel sgemm is hot
    np.dot(
        np.ones((64, D), np.float32), np.ones((64, D), np.float32).T
    )
except Exception:
    pass


def _load_sgemm():
    """Fortran sgemm_ via ctypes: writes C with an arbitrary leading
    dimension, so the host GEMM can target the strided logits view directly
    (np.dot needs a contiguous out + an extra copy pass). Self-tested on the
    exact strided-C pattern; returns None on any failure."""
    import ctypes
    import ctypes.util

    cands = []
    try:
        import re
        import subprocess

        import numpy._core._multiarray_umath as _mu

        out = subprocess.run(
            ["ldd", _mu.__file__], capture_output=True, text=True
        ).stdout
        cands += re.findall(r"=> (\S*(?:blas)\S*)", out)
    except Exception:
        pass
    for n in ("blas", "openblas", "cblas"):
        p = ctypes.util.find_library(n)
        if p:
            cands.append(p)
    cands += ["libblas.so.3", "libopenblas.so.0"]
    for cand in cands:
        try:
            fn = ctypes.CDLL(cand).sgemm_
            m, n, k, ld = 7, 5, 4, 13
            a = np.random.rand(m, k).astype(np.float32)
            b = np.random.rand(n, k).astype(np.float32)
            c = np.zeros((n, ld), np.float32)
            ci = lambda v: ctypes.byref(ctypes.c_int(v))
            cf = lambda v: ctypes.byref(ctypes.c_float(v))
            fn(
                ctypes.c_char_p(b"T"), ctypes.c_char_p(b"N"),
                ci(m), ci(n), ci(k), cf(1.0),
                a.ctypes.data_as(ctypes.c_void_p), ci(k),
                b.ctypes.data_as(ctypes.c_void_p), ci(k), cf(0.0),
                ctypes.c_void_p(c.ctypes.data + 2 * 4), ci(ld),
            )
            if np.abs(c[:, 2 : 2 + m] - b @ a.T).max() < 1e-4 and (
                np.abs(c[:, :2]).max() == 0.0
            ):
                return fn, ctypes
        except Exception:
            continue
    return None


_SGEMM = _load_sgemm()
NW = 4                    # logits download waves (must match device)
WCOLS = VSH_D // NW       # columns per wave per core

# preallocate + pre-touch the big host buffers so kernel() pays no page faults
_LOGITS_BUF = np.empty((TOK, V), np.float32)
_LOGITS_BUF[:: 512 // 8].fill(0.0)  # touch every 4KB page (rows are 128KB)
_LOGITS_BUF.fill(0.0)
_WH_BUF = np.empty((V_HOST, D), np.float32)
_WH_BUF.fill(0.0)
_X32_BUF = np.empty((TOK, D), np.float32)
_GEMM_TMP = np.empty(TOK * 2048, np.float32)
_GEMM_TMP.fill(0.0)


def kernel(_trace=False, **inputs):
    global LAST_EXEC_NS
    import time as _time

    _t0 = _time.perf_counter()
    nc = _get_nc()
    maps_a = _prep_a(inputs)
    TIMINGS["prep_a"] = _time.perf_counter() - _t0
    try:
        _t0 = _time.perf_counter()
        outs = _run_spmd_staged(nc, maps_a, lambda: _prep_b(inputs))
        TIMINGS["dispatch"] = _time.perf_counter() - _t0

        # enqueue the xnorm transfer first — it gates the host GEMM
        xdata = outs["xnorm"].addressable_shards[0].data
        xdata.copy_to_host_async()

        # dequantize the host-side LM rows while the device uploads/executes
        _t0 = _time.perf_counter()
        wh = _WH_BUF
        wh[...] = _LM_HOST["t"]                     # int32 -> f32 widen
        whv = wh.reshape(-1, GS)
        np.multiply(whv, _LM_HOST["s"].reshape(-1, 1), out=whv)
        TIMINGS["lm_deq"] = _time.perf_counter() - _t0

        def _shards(name):
            return [
                s.data
                for s in sorted(
                    outs[name].addressable_shards,
                    key=lambda s: s.index[0].start or 0,
                )
            ]

        sdatas = _shards("logscl")
        waves = [_shards(f"logitsq{w}") for w in range(NW)]

        # blocks until device exec completes (4MB, one shard)
        _t0 = _time.perf_counter()
        xh = np.asarray(xdata)                      # [TOK, D] f16
        # restart the wire with the minimum enqueues (scales + wave 0), widen
        # x while those stream, then enqueue the remaining waves
        for d in sdatas:
            d.copy_to_host_async()
        for d in waves[0]:
            d.copy_to_host_async()
        TIMINGS["xn_wait"] = _time.perf_counter() - _t0

        _t0 = _time.perf_counter()
        x32 = _X32_BUF
        x32[...] = xh                               # f16 -> f32
        for wv in waves[1:]:
            for d in wv:
                d.copy_to_host_async()  # streams in C-land during the GEMM below

        logits = _LOGITS_BUF
        scl_box = []

        def _widen(w):
            if not scl_box:
                scl_box.append([np.asarray(d).reshape(VSH_D) for d in sdatas])
            scl = scl_box[0]
            for c, d in enumerate(waves[w]):
                a = np.asarray(d)  # [TOK, WCOLS] i8
                c0 = c * VSH_D + w * WCOLS
                np.multiply(
                    a, scl[c][w * WCOLS : (w + 1) * WCOLS],
                    out=logits[:, c0 : c0 + WCOLS],
                )

        # one worker thread widens waves as the wire delivers them (asarray
        # blocks with the GIL released); the GEMM runs uninterrupted here
        from concurrent.futures import ThreadPoolExecutor

        wex = ThreadPoolExecutor(1)
        wfuts = [wex.submit(_widen, w) for w in range(NW)]
        if _SGEMM is not None:
            # one sgemm straight into the strided logits view (ldc = V);
            # ctypes releases the GIL so the widen worker keeps running
            fn, ct = _SGEMM
            ci = lambda v: ct.byref(ct.c_int(v))
            cf = lambda v: ct.byref(ct.c_float(v))
            fn(
                ct.c_char_p(b"T"), ct.c_char_p(b"N"),
                ci(V_HOST), ci(TOK), ci(D), cf(1.0),
                wh.ctypes.data_as(ct.c_void_p), ci(D),
                x32.ctypes.data_as(ct.c_void_p), ci(D), cf(0.0),
                ct.c_void_p(logits.ctypes.data + V_DEV * 4), ci(V),
            )
        else:
            blk = 2048
            for v0 in range(0, V_HOST, blk):
                bw = min(blk, V_HOST - v0)
                tmp = _GEMM_TMP[: TOK * bw].reshape(TOK, bw)
                np.dot(x32, wh[v0 : v0 + bw].T, out=tmp)
                logits[:, V_DEV + v0 : V_DEV + v0 + bw] = tmp
        TIMINGS["gemm"] = _time.perf_counter() - _t0
        for f in wfuts:
            f.result()
        wex.shutdown()
        TIMINGS["postproc"] = _time.perf_counter() - _t0
        import os as _os

        if _os.environ.get("KBENCH"):
            print("KERNEL TIMINGS:", {k: round(v, 4) for k, v in TIMINGS.items()})
        return logits.reshape(B, S, V)
    except Exception:
        import os as _os

        if _os.environ.get("KBENCH"):
            import traceback as _tb

            _tb.print_exc()
        maps_b = _prep_b(inputs)
        in_maps = [dict(a, **b) for a, b in zip(maps_a, maps_b)]
        res = run_bass_kernel_spmd(
            nc, in_maps, list(range(N_CORES)), trace=bool(_trace)
        )
        if getattr(res, "exec_time_ns", None):
            LAST_EXEC_NS = res.exec_time_ns
        lt = np.concatenate(
            [
                np.asarray(res.results[c][f"logitsq{w}"])
                for c in range(N_CORES)
                for w in range(NW)
            ],
            axis=1,
        )  # [TOK, V_DEV] token-major
        sc = np.concatenate(
            [np.asarray(res.results[c]["logscl"]) for c in range(N_CORES)],
            axis=0,
        ).reshape(1, V_DEV)
        wh = _LM_HOST["t"].astype(np.float32)
        whv = wh.reshape(-1, GS)
        np.multiply(whv, _LM_HOST["s"].reshape(-1, 1), out=whv)
        x32 = np.asarray(res.results[0]["xnorm"]).astype(np.float32)
        logits = np.empty((TOK, V), np.float32)
        logits[:, :V_DEV] = lt.astype(np.float32) * sc
        logits[:, V_DEV:] = x32 @ wh.T
    return logits.reshape(B, S, V)


# ---------------------------------------------------------- import-time warmup


def _warmup():
    """Compile + load everything once at import with zero-filled inputs so the
    first real kernel() call only pays data transfer and execution."""
    nc = _get_nc()
    pname = nc.partition_id_tensor.name if nc.partition_id_tensor else None
    zm = {}
    for alloc in nc.m.functions[0].allocations:
        if (
            isinstance(alloc, mybir.MemoryLocationSet)
            and alloc.kind == "ExternalInput"
        ):
            name = alloc.memorylocations[0].name
            if name != pname:
                zm[name] = np.zeros(
                    tuple(alloc.tensor_shape), mybir.dt.np(alloc.dtype)
                )
    za = {k: v for k, v in zm.items() if k not in GROUP_B}
    zb = {k: v for k, v in zm.items() if k in GROUP_B}
    outs = _run_spmd_staged(nc, [za] * N_CORES, lambda: [zb] * N_CORES)
    for a in outs.values():
        a.block_until_ready()


try:
    _warmup()
except Exception:
    _NC_CACHE = None
    _SPMD_CACHE = None

